# revision 11
# baseline (speedup 1.0000x reference)
"""Trainium2 Bass kernel for GQA attention with RoPE (dense_transformer).

Model: B=2, T=2048, C=2048, H=16 query heads, KV=4 kv heads, D=128, causal.
Sharding: 8 cores = batch(2) x kv-group(4) tensor parallel. Each core computes
its batch's 4 query heads (one kv head), then a partial output projection over
its 512 head-dims; a 4-core ReduceScatter sums the partials and leaves each
core with a 512-row stripe of the final [2048, 2048] output.
"""

import os

os.environ.setdefault("MYCRO_LOCAL_CACHE", "1")

import numpy as np

B, T, C = 2, 2048, 2048
H, KV, D = 16, 4, 128
HL = H // KV          # 4 local query heads per core
NCORES = 8
P = 128
SCALE = 1.0 / float(np.sqrt(D))

NT = T // P           # 16 t-blocks
NCC = C // P          # 16 c-chunks
NTC = T // 512        # 4 t-chunks of 512
NG = T // 512         # 4 q-block groups (512 queries each)
TQ = 512              # queries per attention group
NEG = -1e10


def _emit(nc, tile, mybir, ExitStack):
    from concourse.masks import make_identity

    f32 = mybir.dt.float32
    bf16 = mybir.dt.bfloat16
    Exp = mybir.ActivationFunctionType.Exp
    Copy = mybir.ActivationFunctionType.Copy
    add = mybir.AluOpType.add

    xb = nc.dram_tensor("xb", [T, C], f32, kind="ExternalInput")
    wq = nc.dram_tensor("wq", [HL * D, C], f32, kind="ExternalInput")
    wk = nc.dram_tensor("wk", [D, C], f32, kind="ExternalInput")
    wv = nc.dram_tensor("wv", [D, C], f32, kind="ExternalInput")
    wo = nc.dram_tensor("wo", [C, HL * D], f32, kind="ExternalInput")
    fcos = nc.dram_tensor("fcos", [T, D // 2], f32, kind="ExternalInput")
    fsin = nc.dram_tensor("fsin", [T, D // 2], f32, kind="ExternalInput")
    out = nc.dram_tensor("out", [T // 4, C], f32, kind="ExternalOutput")

    te, ve, sc, gp, sy = nc.tensor, nc.vector, nc.scalar, nc.gpsimd, nc.sync

    with tile.TileContext(nc) as tc, ExitStack() as ctx:
        consts = ctx.enter_context(tc.tile_pool(name="consts", bufs=1))
        persist = ctx.enter_context(tc.tile_pool(name="persist", bufs=1))
        dram = ctx.enter_context(tc.tile_pool(name="dram", bufs=1, space="DRAM"))

        ident = consts.tile([P, P], bf16, tag="ident")
        make_identity(nc, ident[:])
        # scoresT layout [tk, tq]: keep where tq >= tk, else -1e10.
        triT = consts.tile([P, P], f32, tag="triT")
        gp.memset(triT[:], 0.0)
        gp.affine_select(
            out=triT[:], in_=triT[:], compare_op=mybir.AluOpType.is_ge,
            fill=NEG, base=0, pattern=[[1, P]], channel_multiplier=-1,
        )
        ones = consts.tile([P, 1], bf16, tag="ones")
        gp.memset(ones[:], 1.0)
        onesc = consts.tile([P, P], f32, tag="onesc")
        gp.memset(onesc[:], 1.0)

        # tensors that live from projection until the end
        qrT = [persist.tile([P, T], bf16, tag=f"qrT{h}", name=f"qrT{h}")
               for h in range(HL)]
        krT = persist.tile([P, T], bf16, tag="krT")
        vnat = persist.tile([P, T], bf16, tag="vnat")
        attnT = [persist.tile([P, T], bf16, tag=f"attnT{h}", name=f"attnT{h}")
                 for h in range(HL)]

        y_dram = dram.tile([T, C], f32, tag="ydram")
        rs_out = dram.tile([T // 4, C], f32, tag="rsout")

        # ---- projection phase (pools freed afterwards) ------------------------
        with tc.tile_pool(name="proj", bufs=1) as proj:
            cosq = proj.tile([P, T], bf16, tag="cosq")
            sinq = proj.tile([P, T], bf16, tag="sinq")
            cosk = proj.tile([P, T], bf16, tag="cosk")
            sink = proj.tile([P, T], bf16, tag="sink")
            xT = [proj.tile([P, T], bf16, tag=f"xT{cc}", name=f"xT{cc}")
                  for cc in range(NCC)]
            wqT = [proj.tile([P, HL * P], bf16, tag=f"wqT{cc}", name=f"wqT{cc}")
                   for cc in range(NCC)]
            wkT = [proj.tile([P, P], bf16, tag=f"wkT{cc}", name=f"wkT{cc}")
                   for cc in range(NCC)]
            wvT = [proj.tile([P, P], bf16, tag=f"wvT{cc}", name=f"wvT{cc}")
                   for cc in range(NCC)]
            vT = proj.tile([P, T], bf16, tag="vT")

            # -- stage A: loads, casts, transposes --
            with tc.tile_pool(name="stA", bufs=2) as sbA, \
                 tc.tile_pool(name="stAbf", bufs=4) as sbX, \
                 tc.tile_pool(name="psA", bufs=2, space="PSUM") as psA:

                # freqs -> cosT/sinT (bf16); q-versions pre-scaled by 1/sqrt(D)
                for src, dq, dk in ((fcos, cosq, cosk), (fsin, sinq, sink)):
                    for tb in range(NT):
                        ft = sbA.tile([P, 64], f32, tag="frq_in", name="frq_in")
                        sy.dma_start(ft[:], src.ap()[tb * P:(tb + 1) * P, :])
                        fb = sbA.tile([P, 64], bf16, tag="frq_bf", name="frq_bf")
                        ve.tensor_copy(fb[:], ft[:])
                        pf = psA.tile([64, P], bf16, tag="frq_ps", name="frq_ps")
                        te.transpose(pf[:], fb[:], ident[:])
                        sc.activation(dq[0:64, tb * P:(tb + 1) * P], pf[:],
                                      Copy, scale=SCALE)
                        sc.activation(dk[0:64, tb * P:(tb + 1) * P], pf[:],
                                      Copy)
                    # duplicate into the upper partition half
                    sy.dma_start(dq[64:P, :], dq[0:64, :])
                    sy.dma_start(dk[64:P, :], dk[0:64, :])

                # x -> xT (bf16)  [c-chunk][c_in_chunk, t]
                for tbg in range(NT // 4):
                    xbf = []
                    for i in range(4):
                        tb = tbg * 4 + i
                        xt = sbA.tile([P, C], f32, tag="big_in", name="big_in")
                        sy.dma_start(xt[:], xb.ap()[tb * P:(tb + 1) * P, :])
                        xc = sbX.tile([P, C], bf16, tag="big_bf", name="big_bf")
                        ve.tensor_copy(xc[:], xt[:])
                        xbf.append(xc)
                    for cc in range(NCC):
                        ps = psA.tile([P, 512], bf16, tag="ptr", name="ptr")
                        for i in range(4):
                            te.transpose(
                                ps[:, i * P:(i + 1) * P],
                                xbf[i][:, cc * P:(cc + 1) * P], ident[:],
                            )
                        sc.activation(
                            xT[cc][:, tbg * 512:(tbg + 1) * 512], ps[:], Copy)

                # wq (rows permuted even/odd per head) -> wqT
                wq_eo = wq.ap().rearrange("(a two) c -> two a c", two=2)
                wq_bf = []
                for h in range(HL):
                    wt = sbA.tile([P, C], f32, tag="big_in", name="big_in")
                    sy.dma_start(wt[0:64, :], wq_eo[0, h * 64:(h + 1) * 64, :])
                    sy.dma_start(wt[64:P, :], wq_eo[1, h * 64:(h + 1) * 64, :])
                    wb = sbX.tile([P, C], bf16, tag="big_bf", name="big_bf")
                    ve.tensor_copy(wb[:], wt[:])
                    wq_bf.append(wb)
                for cc in range(NCC):
                    ps = psA.tile([P, 512], bf16, tag="ptr", name="ptr")
                    for h in range(HL):
                        te.transpose(
                            ps[:, h * P:(h + 1) * P],
                            wq_bf[h][:, cc * P:(cc + 1) * P], ident[:],
                        )
                    sc.activation(wqT[cc][:], ps[:], Copy)

                # wk (permuted) / wv (natural) -> wkT / wvT
                wk_eo = wk.ap().rearrange("(a two) c -> two a c", two=2)
                for src_eo, src, dst, perm in (
                        (wk_eo, wk, wkT, True), (None, wv, wvT, False)):
                    wt = sbA.tile([P, C], f32, tag="big_in", name="big_in")
                    if perm:
                        sy.dma_start(wt[0:64, :], src_eo[0, :, :])
                        sy.dma_start(wt[64:P, :], src_eo[1, :, :])
                    else:
                        sy.dma_start(wt[:], src.ap()[:, :])
                    wb = sbX.tile([P, C], bf16, tag="big_bf", name="big_bf")
                    ve.tensor_copy(wb[:], wt[:])
                    for ccg in range(NCC // 4):
                        ps = psA.tile([P, 512], bf16, tag="ptr", name="ptr")
                        for i in range(4):
                            cc = ccg * 4 + i
                            te.transpose(
                                ps[:, i * P:(i + 1) * P],
                                wb[:, cc * P:(cc + 1) * P], ident[:],
                            )
                        for i in range(4):
                            cc = ccg * 4 + i
                            sc.activation(dst[cc][:], ps[:, i * P:(i + 1) * P],
                                          Copy)

            # -- stage B: QKV projections + rope --
            with tc.tile_pool(name="stB", bufs=4) as sbB, \
                 tc.tile_pool(name="ropetmp", bufs=8) as sbR, \
                 tc.tile_pool(name="psB", bufs=2, space="PSUM") as psB:

                def qkv_unit(weight_slices, dst, rope, cos_t, sin_t):
                    for tc4 in range(NTC):
                        ps = psB.tile([P, 512], f32, tag="pqkv", name="pqkv")
                        for cc in range(NCC):
                            te.matmul(
                                ps[:], weight_slices[cc],
                                xT[cc][:, tc4 * 512:(tc4 + 1) * 512],
                                start=(cc == 0), stop=(cc == NCC - 1),
                            )
                        sl = slice(tc4 * 512, (tc4 + 1) * 512)
                        if not rope:
                            sc.activation(dst[:, sl], ps[:], Copy)
                            continue
                        qs = sbB.tile([P, 512], bf16, tag="qkev", name="qkev")
                        sc.activation(qs[:], ps[:], Copy)
                        # realign halves so every DVE op is base-consistent
                        q1lo = sbR.tile([64, 512], bf16, tag="q1lo", name="q1lo")
                        sy.dma_start(q1lo[:], qs[64:P, :])
                        q0hi = sbR.tile([P, 512], bf16, tag="q0hi", name="q0hi")
                        sy.dma_start(q0hi[64:P, :], qs[0:64, :])
                        ta = sbR.tile([64, 512], bf16, tag="rta", name="rta")
                        tb2 = sbR.tile([64, 512], bf16, tag="rtb", name="rtb")
                        ve.tensor_mul(ta[:], qs[0:64, :], cos_t[0:64, sl])
                        ve.tensor_mul(tb2[:], q1lo[:], sin_t[0:64, sl])
                        ve.tensor_sub(dst[0:64, sl], ta[:], tb2[:])
                        tc2 = sbR.tile([P, 512], bf16, tag="rtc", name="rtc")
                        td = sbR.tile([P, 512], bf16, tag="rtd", name="rtd")
                        ve.tensor_mul(tc2[64:P, :], q0hi[64:P, :], sin_t[64:P, sl])
                        ve.tensor_mul(td[64:P, :], qs[64:P, :], cos_t[64:P, sl])
                        ve.tensor_add(dst[64:P, sl], tc2[64:P, :], td[64:P, :])

                for h in range(HL):
                    qkv_unit(
                        [wqT[cc][:, h * P:(h + 1) * P] for cc in range(NCC)],
                        qrT[h], True, cosq, sinq,
                    )
                qkv_unit([wkT[cc][:] for cc in range(NCC)], krT, True,
                         cosk, sink)
                qkv_unit([wvT[cc][:] for cc in range(NCC)], vT, False,
                         None, None)

                # vT [dv, t] -> vnat [partition=t%128, col = kb*128 + dv]
                for tbg in range(NT // 4):
                    ps = psB.tile([P, 512], bf16, tag="pvtr", name="pvtr")
                    for i in range(4):
                        tb = tbg * 4 + i
                        te.transpose(
                            ps[:, i * P:(i + 1) * P],
                            vT[:, tb * P:(tb + 1) * P], ident[:],
                        )
                    sc.activation(vnat[:, tbg * 512:(tbg + 1) * 512], ps[:],
                                  Copy)

        # ---- attention + output projection phase ------------------------------
        with tc.tile_pool(name="stCD", bufs=1) as sbCD, \
             tc.tile_pool(name="stCDin", bufs=2) as sbCin, \
             tc.tile_pool(name="stCDbf", bufs=4) as sbCbf, \
             tc.tile_pool(name="probsP", bufs=6) as sbP, \
             tc.tile_pool(name="stC", bufs=2) as sbC, \
             tc.tile_pool(name="stCbc", bufs=4) as sbBC, \
             tc.tile_pool(name="stD", bufs=3) as sbD, \
             tc.tile_pool(name="psWo", bufs=1, space="PSUM") as psWo, \
             tc.tile_pool(name="psT", bufs=2, space="PSUM") as psT, \
             tc.tile_pool(name="psAttn", bufs=2, space="PSUM") as psAt, \
             tc.tile_pool(name="psSums", bufs=1, space="PSUM") as psSm, \
             tc.tile_pool(name="psY", bufs=1, space="PSUM") as psY, \
             tc.tile_pool(name="psBC", bufs=1, space="PSUM") as psBC:

            # wo [C, HL*D] -> woT[h] [dv, C]  (overlaps attention on PE gaps)
            woT = [sbCD.tile([P, C], bf16, tag=f"woT{h}", name=f"woT{h}")
                   for h in range(HL)]
            for ctg in range(NCC // 4):
                wo_bf = []
                for i in range(4):
                    ct = ctg * 4 + i
                    wt = sbCin.tile([P, HL * P], f32, tag="wo_in", name="wo_in")
                    sy.dma_start(wt[:], wo.ap()[ct * P:(ct + 1) * P, :])
                    wb = sbCbf.tile([P, HL * P], bf16, tag="wo_bf", name="wo_bf")
                    ve.tensor_copy(wb[:], wt[:])
                    wo_bf.append(wb)
                for h in range(HL):
                    ps = psWo.tile([P, 512], bf16, tag="pwo", name="pwo")
                    for i in range(4):
                        te.transpose(
                            ps[:, i * P:(i + 1) * P],
                            wo_bf[i][:, h * P:(h + 1) * P], ident[:],
                        )
                    sc.activation(woT[h][:, ctg * 512:(ctg + 1) * 512], ps[:],
                                  Copy)

            # attention, two heads per sweep to limit PSUM pressure
            for gq in range(NG):
                kbmax = 4 * (gq + 1)
                qsl = slice(gq * TQ, (gq + 1) * TQ)
                for hp in range(HL // 2):
                    hs = (2 * hp, 2 * hp + 1)
                    pa = [psAt.tile([P, TQ], f32, tag="pattn", name="pattn")
                          for _ in hs]
                    psums = psSm.tile([P, TQ], f32, tag="psums", name="psums")
                    for kb in range(kbmax):
                        j = kb - 4 * gq  # >= 0 on diagonal blocks
                        w0 = max(j, 0) * P
                        probs = []
                        for h in hs:
                            st = psT.tile([P, TQ], f32, tag="pscore",
                                          name="pscore")
                            te.matmul(
                                st[:, w0:TQ],
                                krT[:, kb * P:(kb + 1) * P],
                                qrT[h][:, gq * TQ + w0:(gq + 1) * TQ],
                                start=True, stop=True,
                            )
                            if j >= 0:
                                ve.tensor_tensor(
                                    st[:, w0:w0 + P], st[:, w0:w0 + P],
                                    triT[:], add)
                            pb = sbP.tile([P, TQ], bf16, tag="probs",
                                          name="probs")
                            sc.activation(pb[:, w0:TQ], st[:, w0:TQ], Exp)
                            if w0 > 0:
                                gp.memset(pb[:, 0:w0], 0.0)
                            probs.append(pb)
                        for i in range(2):
                            te.matmul(
                                psums[64 * i:64 * i + 1, :], ones[:],
                                probs[i][:],
                                start=(kb == 0), stop=(kb == kbmax - 1),
                            )
                        for i in range(2):
                            te.matmul(
                                pa[i][:], vnat[:, kb * P:(kb + 1) * P],
                                probs[i][:],
                                start=(kb == 0), stop=(kb == kbmax - 1),
                            )
                    recip = sbC.tile([P, TQ], f32, tag="recip", name="recip")
                    ve.reciprocal(recip[0:1, :], psums[0:1, :])
                    ve.reciprocal(recip[64:65, :], psums[64:65, :])
                    for i, h in enumerate(hs):
                        pbc = psBC.tile([P, TQ], f32, tag="pbc", name="pbc")
                        te.matmul(pbc[:], onesc[64 * i:64 * i + 1, 0:P],
                                  recip[64 * i:64 * i + 1, :],
                                  start=True, stop=True)
                        bc = sbBC.tile([P, TQ], f32, tag="rbc", name="rbc")
                        sc.activation(bc[:], pbc[:], Copy)
                        ve.tensor_mul(attnT[h][:, qsl], pa[i][:], bc[:])

            # -- output projection (overlaps attention tail) --
            for tb in range(NT):
                ysb = sbD.tile([P, C], f32, tag="ysb", name="ysb")
                for cc4 in range(C // 512):
                    py = psY.tile([P, 512], f32, tag="py", name="py")
                    for h in range(HL):
                        te.matmul(
                            py[:],
                            attnT[h][:, tb * P:(tb + 1) * P],
                            woT[h][:, cc4 * 512:(cc4 + 1) * 512],
                            start=(h == 0), stop=(h == HL - 1),
                        )
                    ve.tensor_copy(ysb[:, cc4 * 512:(cc4 + 1) * 512], py[:])
                sy.dma_start(y_dram[tb * P:(tb + 1) * P, :], ysb[:])

        # ---- reduce-scatter across the 4 cores of this batch ------------------
        gp.collective_compute(
            "ReduceScatter", mybir.AluOpType.add,
            replica_groups=[[0, 1, 2, 3], [4, 5, 6, 7]],
            ins=[y_dram.opt()], outs=[rs_out.opt()],
        )
        sy.dma_start(out.ap()[:, :], rs_out[:])

    return nc


_PROGRAM = None


def _get_program():
    global _PROGRAM
    if _PROGRAM is None:
        from contextlib import ExitStack
        import concourse.tile as tile
        from concourse import bacc, mybir

        nc = bacc.Bacc("TRN2", target_bir_lowering=False, debug=False,
                       num_devices=NCORES)
        _emit(nc, tile, mybir, ExitStack)
        nc.compile()
        _PROGRAM = nc
    return _PROGRAM


def kernel(x, wq, wk, wv, wo, freqs_cos, freqs_sin, mask=None):
    from concourse.bass_utils import run_bass_kernel_spmd

    x = np.asarray(x, np.float32)
    wq = np.asarray(wq, np.float32)
    wk = np.asarray(wk, np.float32)
    wv = np.asarray(wv, np.float32)
    wo = np.asarray(wo, np.float32)
    fc = np.ascontiguousarray(np.asarray(freqs_cos, np.float32))
    fs = np.ascontiguousarray(np.asarray(freqs_sin, np.float32))

    nc = _get_program()
    in_maps = []
    for core in range(NCORES):
        b, g = core // 4, core % 4
        in_maps.append({
            "xb": np.ascontiguousarray(x[b]),
            "wq": np.ascontiguousarray(wq[g * HL * D:(g + 1) * HL * D]),
            "wk": np.ascontiguousarray(wk[g * D:(g + 1) * D]),
            "wv": np.ascontiguousarray(wv[g * D:(g + 1) * D]),
            "wo": np.ascontiguousarray(wo[:, g * HL * D:(g + 1) * HL * D]),
            "fcos": fc,
            "fsin": fs,
        })
    res = run_bass_kernel_spmd(nc, in_maps, core_ids=list(range(NCORES)))
    outp = np.empty((B, T, C), np.float32)
    stripe = T // 4
    for b in range(B):
        for r in range(4):
            outp[b, r * stripe:(r + 1) * stripe] = res.results[4 * b + r]["out"]
    return outp


# revision 16
# speedup vs baseline: 1.1602x; 1.1602x over previous
"""Trainium2 Bass kernel for GQA attention with RoPE (dense_transformer).

Model: B=2, T=2048, C=2048, H=16 query heads, KV=4 kv heads, D=128, causal.
Sharding: 8 cores = batch(2) x kv-group(4) tensor parallel. Each core computes
its batch's 4 query heads (one kv head), then a partial output projection over
its 512 head-dims; a 4-core ReduceScatter sums the partials and leaves each
core with a 512-row stripe of the final [2048, 2048] output.
"""

import os

os.environ.setdefault("MYCRO_LOCAL_CACHE", "1")

import numpy as np

B, T, C = 2, 2048, 2048
H, KV, D = 16, 4, 128
HL = H // KV          # 4 local query heads per core
NCORES = 8
P = 128
SCALE = 1.0 / float(np.sqrt(D))

NT = T // P           # 16 t-blocks
NCC = C // P          # 16 c-chunks
NTC = T // 512        # 4 t-chunks of 512
NG = T // 512         # 4 q-block groups (512 queries each)
TQ = 512              # queries per attention group
NEG = -1e10


def _emit(nc, tile, mybir, ExitStack):
    from concourse.masks import make_identity

    f32 = mybir.dt.float32
    f32r = mybir.dt.float32r
    bf16 = mybir.dt.bfloat16
    Exp = mybir.ActivationFunctionType.Exp
    Copy = mybir.ActivationFunctionType.Copy
    add = mybir.AluOpType.add

    xb = nc.dram_tensor("xb", [T, C], f32, kind="ExternalInput")
    wq = nc.dram_tensor("wq", [HL * D, C], f32, kind="ExternalInput")
    wk = nc.dram_tensor("wk", [D, C], f32, kind="ExternalInput")
    wv = nc.dram_tensor("wv", [D, C], f32, kind="ExternalInput")
    wo = nc.dram_tensor("wo", [C, HL * D], f32, kind="ExternalInput")
    fcos = nc.dram_tensor("fcos", [T, D // 2], f32, kind="ExternalInput")
    fsin = nc.dram_tensor("fsin", [T, D // 2], f32, kind="ExternalInput")
    out = nc.dram_tensor("out", [T // 4, C], f32, kind="ExternalOutput")

    te, ve, sc, gp, sy = nc.tensor, nc.vector, nc.scalar, nc.gpsimd, nc.sync

    with tile.TileContext(nc) as tc, ExitStack() as ctx:
        consts = ctx.enter_context(tc.tile_pool(name="consts", bufs=1))
        persist = ctx.enter_context(tc.tile_pool(name="persist", bufs=1))
        dram = ctx.enter_context(tc.tile_pool(name="dram", bufs=1, space="DRAM"))

        ident = consts.tile([P, P], bf16, tag="ident")
        make_identity(nc, ident[:])
        # scoresT layout [tk, tq]: keep where tq >= tk, else -1e10.
        triT = consts.tile([P, P], f32, tag="triT")
        gp.memset(triT[:], 0.0)
        gp.affine_select(
            out=triT[:], in_=triT[:], compare_op=mybir.AluOpType.is_ge,
            fill=NEG, base=0, pattern=[[1, P]], channel_multiplier=-1,
        )
        ones = consts.tile([P, 1], bf16, tag="ones")
        gp.memset(ones[:], 1.0)
        onesc = consts.tile([P, P], f32, tag="onesc")
        gp.memset(onesc[:], 1.0)

        # tensors that live from projection until the end
        qrT = [persist.tile([P, T], bf16, tag=f"qrT{h}", name=f"qrT{h}")
               for h in range(HL)]
        krT = persist.tile([P, T], bf16, tag="krT")
        vnat = persist.tile([P, T], bf16, tag="vnat")
        attnT = [persist.tile([P, T], bf16, tag=f"attnT{h}", name=f"attnT{h}")
                 for h in range(HL)]

        y_dram = dram.tile([T, C], f32, tag="ydram")
        rs_out = [dram.tile([P, C], f32, tag=f"rsout{g}", name=f"rsout{g}")
                  for g in range(NG)]

        # ---- projection phase (pools freed afterwards) ------------------------
        with tc.tile_pool(name="proj", bufs=1) as proj:
            cosq = proj.tile([P, T], bf16, tag="cosq")
            sinq = proj.tile([P, T], bf16, tag="sinq")
            cosk = proj.tile([P, T], bf16, tag="cosk")
            sink = proj.tile([P, T], bf16, tag="sink")
            xT = [proj.tile([P, T], bf16, tag=f"xT{cc}", name=f"xT{cc}")
                  for cc in range(NCC)]
            wqT = [proj.tile([P, HL * P], bf16, tag=f"wqT{cc}", name=f"wqT{cc}")
                   for cc in range(NCC)]
            wkT = [proj.tile([P, P], bf16, tag=f"wkT{cc}", name=f"wkT{cc}")
                   for cc in range(NCC)]
            wvT = [proj.tile([P, P], bf16, tag=f"wvT{cc}", name=f"wvT{cc}")
                   for cc in range(NCC)]
            vT = proj.tile([P, T], bf16, tag="vT")

            # -- stage A: loads, casts, transposes --
            with tc.tile_pool(name="stA", bufs=2) as sbA, \
                 tc.tile_pool(name="stAbf", bufs=4) as sbX, \
                 tc.tile_pool(name="psA", bufs=2, space="PSUM") as psA:

                # freqs -> cosT/sinT (bf16); q-versions pre-scaled by 1/sqrt(D)
                for src, dq, dk in ((fcos, cosq, cosk), (fsin, sinq, sink)):
                    for tb in range(NT):
                        ft = sbA.tile([P, 64], f32, tag="frq_in", name="frq_in")
                        sc.dma_start(ft[:], src.ap()[tb * P:(tb + 1) * P, :])
                        fb = sbA.tile([P, 64], bf16, tag="frq_bf", name="frq_bf")
                        ve.tensor_copy(fb[:], ft[:])
                        pf = psA.tile([64, P], bf16, tag="frq_ps", name="frq_ps")
                        te.transpose(pf[:], fb[:], ident[:])
                        sc.activation(dq[0:64, tb * P:(tb + 1) * P], pf[:],
                                      Copy, scale=SCALE)
                        sc.activation(dk[0:64, tb * P:(tb + 1) * P], pf[:],
                                      Copy)
                    # duplicate into the upper partition half
                    sy.dma_start(dq[64:P, :], dq[0:64, :])
                    sy.dma_start(dk[64:P, :], dk[0:64, :])

                # x -> xT (bf16)  [c-chunk][c_in_chunk, t]
                for tbg in range(NT // 4):
                    xbf = []
                    for i in range(4):
                        tb = tbg * 4 + i
                        xt = sbA.tile([P, C], f32, tag="big_in", name="big_in")
                        sy.dma_start(xt[:], xb.ap()[tb * P:(tb + 1) * P, :])
                        xc = sbX.tile([P, C], bf16, tag="big_bf", name="big_bf")
                        ve.tensor_copy(xc[:], xt[:])
                        xbf.append(xc)
                    for cc in range(NCC):
                        ps = psA.tile([P, 512], bf16, tag="ptr", name="ptr")
                        for i in range(4):
                            te.transpose(
                                ps[:, i * P:(i + 1) * P],
                                xbf[i][:, cc * P:(cc + 1) * P], ident[:],
                            )
                        sc.activation(
                            xT[cc][:, tbg * 512:(tbg + 1) * 512], ps[:], Copy)

                # wq (rows permuted even/odd per head) -> wqT
                wq_eo = wq.ap().rearrange("(a two) c -> two a c", two=2)
                wq_bf = []
                for h in range(HL):
                    wt = sbA.tile([P, C], f32, tag="big_in", name="big_in")
                    gp.dma_start(wt[0:64, :], wq_eo[0, h * 64:(h + 1) * 64, :])
                    gp.dma_start(wt[64:P, :], wq_eo[1, h * 64:(h + 1) * 64, :])
                    wb = sbX.tile([P, C], bf16, tag="big_bf", name="big_bf")
                    ve.tensor_copy(wb[:], wt[:])
                    wq_bf.append(wb)
                for cc in range(NCC):
                    ps = psA.tile([P, 512], bf16, tag="ptr", name="ptr")
                    for h in range(HL):
                        te.transpose(
                            ps[:, h * P:(h + 1) * P],
                            wq_bf[h][:, cc * P:(cc + 1) * P], ident[:],
                        )
                    sc.activation(wqT[cc][:], ps[:], Copy)

                # wk (permuted) / wv (natural) -> wkT / wvT
                wk_eo = wk.ap().rearrange("(a two) c -> two a c", two=2)
                for src_eo, src, dst, perm in (
                        (wk_eo, wk, wkT, True), (None, wv, wvT, False)):
                    wt = sbA.tile([P, C], f32, tag="big_in", name="big_in")
                    if perm:
                        gp.dma_start(wt[0:64, :], src_eo[0, :, :])
                        gp.dma_start(wt[64:P, :], src_eo[1, :, :])
                    else:
                        gp.dma_start(wt[:], src.ap()[:, :])
                    wb = sbX.tile([P, C], bf16, tag="big_bf", name="big_bf")
                    ve.tensor_copy(wb[:], wt[:])
                    for ccg in range(NCC // 4):
                        ps = psA.tile([P, 512], bf16, tag="ptr", name="ptr")
                        for i in range(4):
                            cc = ccg * 4 + i
                            te.transpose(
                                ps[:, i * P:(i + 1) * P],
                                wb[:, cc * P:(cc + 1) * P], ident[:],
                            )
                        for i in range(4):
                            cc = ccg * 4 + i
                            sc.activation(dst[cc][:], ps[:, i * P:(i + 1) * P],
                                          Copy)

            # -- stage B: QKV projections + rope --
            with tc.tile_pool(name="stB", bufs=4) as sbB, \
                 tc.tile_pool(name="ropetmp", bufs=8) as sbR, \
                 tc.tile_pool(name="psB", bufs=2, space="PSUM") as psB:

                def qkv_unit(weight_slices, dst, rope, cos_t, sin_t):
                    for tc4 in range(NTC):
                        ps = psB.tile([P, 512], f32, tag="pqkv", name="pqkv")
                        for cc in range(NCC):
                            te.matmul(
                                ps[:], weight_slices[cc],
                                xT[cc][:, tc4 * 512:(tc4 + 1) * 512],
                                start=(cc == 0), stop=(cc == NCC - 1),
                            )
                        sl = slice(tc4 * 512, (tc4 + 1) * 512)
                        if not rope:
                            sc.activation(dst[:, sl], ps[:], Copy)
                            continue
                        qs = sbB.tile([P, 512], bf16, tag="qkev", name="qkev")
                        sc.activation(qs[:], ps[:], Copy)
                        # realign halves so every DVE op is base-consistent
                        q1lo = sbR.tile([64, 512], bf16, tag="q1lo", name="q1lo")
                        sy.dma_start(q1lo[:], qs[64:P, :])
                        q0hi = sbR.tile([P, 512], bf16, tag="q0hi", name="q0hi")
                        sy.dma_start(q0hi[64:P, :], qs[0:64, :])
                        ta = sbR.tile([64, 512], bf16, tag="rta", name="rta")
                        tb2 = sbR.tile([64, 512], bf16, tag="rtb", name="rtb")
                        ve.tensor_mul(ta[:], qs[0:64, :], cos_t[0:64, sl])
                        ve.tensor_mul(tb2[:], q1lo[:], sin_t[0:64, sl])
                        ve.tensor_sub(dst[0:64, sl], ta[:], tb2[:])
                        tc2 = sbR.tile([P, 512], bf16, tag="rtc", name="rtc")
                        td = sbR.tile([P, 512], bf16, tag="rtd", name="rtd")
                        ve.tensor_mul(tc2[64:P, :], q0hi[64:P, :], sin_t[64:P, sl])
                        ve.tensor_mul(td[64:P, :], qs[64:P, :], cos_t[64:P, sl])
                        ve.tensor_add(dst[64:P, sl], tc2[64:P, :], td[64:P, :])

                for h in range(HL):
                    qkv_unit(
                        [wqT[cc][:, h * P:(h + 1) * P] for cc in range(NCC)],
                        qrT[h], True, cosq, sinq,
                    )
                qkv_unit([wkT[cc][:] for cc in range(NCC)], krT, True,
                         cosk, sink)
                qkv_unit([wvT[cc][:] for cc in range(NCC)], vT, False,
                         None, None)

                # vT [dv, t] -> vnat [partition=t%128, col = kb*128 + dv]
                for tbg in range(NT // 4):
                    ps = psB.tile([P, 512], bf16, tag="pvtr", name="pvtr")
                    for i in range(4):
                        tb = tbg * 4 + i
                        te.transpose(
                            ps[:, i * P:(i + 1) * P],
                            vT[:, tb * P:(tb + 1) * P], ident[:],
                        )
                    sc.activation(vnat[:, tbg * 512:(tbg + 1) * 512], ps[:],
                                  Copy)

        # ---- attention + output projection phase ------------------------------
        with tc.tile_pool(name="stCD", bufs=1) as sbCD, \
             tc.tile_pool(name="stCDin", bufs=2) as sbCin, \
             tc.tile_pool(name="stCDbf", bufs=4) as sbCbf, \
             tc.tile_pool(name="probsP", bufs=6) as sbP, \
             tc.tile_pool(name="stC", bufs=2) as sbC, \
             tc.tile_pool(name="stCbc", bufs=4) as sbBC, \
             tc.tile_pool(name="stD", bufs=3) as sbD, \
             tc.tile_pool(name="psWo", bufs=1, space="PSUM") as psWo, \
             tc.tile_pool(name="psT", bufs=2, space="PSUM") as psT, \
             tc.tile_pool(name="psAttn", bufs=2, space="PSUM") as psAt, \
             tc.tile_pool(name="psSums", bufs=1, space="PSUM") as psSm, \
             tc.tile_pool(name="psY", bufs=1, space="PSUM") as psY, \
             tc.tile_pool(name="psBC", bufs=1, space="PSUM") as psBC:

            # wo [C, HL*D] -> woT[h] [dv, C]  (overlaps attention on PE gaps)
            woT = [sbCD.tile([P, C], bf16, tag=f"woT{h}", name=f"woT{h}")
                   for h in range(HL)]
            for ctg in range(NCC // 4):
                wo_bf = []
                for i in range(4):
                    ct = ctg * 4 + i
                    wt = sbCin.tile([P, HL * P], f32, tag="wo_in", name="wo_in")
                    gp.dma_start(wt[:], wo.ap()[ct * P:(ct + 1) * P, :])
                    wb = sbCbf.tile([P, HL * P], bf16, tag="wo_bf", name="wo_bf")
                    ve.tensor_copy(wb[:], wt[:])
                    wo_bf.append(wb)
                for h in range(HL):
                    ps = psWo.tile([P, 512], bf16, tag="pwo", name="pwo")
                    for i in range(4):
                        te.transpose(
                            ps[:, i * P:(i + 1) * P],
                            wo_bf[i][:, h * P:(h + 1) * P], ident[:],
                        )
                    sc.activation(woT[h][:, ctg * 512:(ctg + 1) * 512], ps[:],
                                  Copy)

            # attention, two heads per sweep to limit PSUM pressure;
            # per-group output projection + reduce-scatter overlap the rest
            for gq in range(NG):
                kbmax = 4 * (gq + 1)
                qsl = slice(gq * TQ, (gq + 1) * TQ)
                for hp in range(HL // 2):
                    hs = (2 * hp, 2 * hp + 1)
                    pa = [psAt.tile([P, TQ], f32, tag="pattn", name="pattn")
                          for _ in hs]
                    psums = psSm.tile([P, TQ], f32, tag="psums", name="psums")
                    for kb in range(kbmax):
                        j = kb - 4 * gq  # >= 0 on diagonal blocks
                        w0 = max(j, 0) * P
                        probs = []
                        for h in hs:
                            st = psT.tile([P, TQ], f32, tag="pscore",
                                          name="pscore")
                            te.matmul(
                                st[:, w0:TQ],
                                krT[:, kb * P:(kb + 1) * P],
                                qrT[h][:, gq * TQ + w0:(gq + 1) * TQ],
                                start=True, stop=True,
                            )
                            if j >= 0:
                                ve.tensor_tensor(
                                    st[:, w0:w0 + P], st[:, w0:w0 + P],
                                    triT[:], add)
                            pb = sbP.tile([P, TQ], bf16, tag="probs",
                                          name="probs")
                            sc.activation(pb[:, w0:TQ], st[:, w0:TQ], Exp)
                            if w0 > 0:
                                gp.memset(pb[:, 0:w0], 0.0)
                            probs.append(pb)
                        for i in range(2):
                            te.matmul(
                                psums[64 * i:64 * i + 1, :], ones[:],
                                probs[i][:],
                                start=(kb == 0), stop=(kb == kbmax - 1),
                            )
                        for i in range(2):
                            te.matmul(
                                pa[i][:], vnat[:, kb * P:(kb + 1) * P],
                                probs[i][:],
                                start=(kb == 0), stop=(kb == kbmax - 1),
                            )
                    # evict unnormalized immediately (frees PSUM for the next
                    # sweep), normalize off the critical path
                    sums_sb = sbC.tile([P, TQ], f32, tag="sums_sb",
                                       name="sums_sb")
                    sc.activation(sums_sb[0:1, :], psums[0:1, :], Copy)
                    sc.activation(sums_sb[64:65, :], psums[64:65, :], Copy)
                    for i, h in enumerate(hs):
                        sc.activation(attnT[h][:, qsl], pa[i][:], Copy)
                    recip = sbC.tile([P, TQ], f32, tag="recip", name="recip")
                    ve.reciprocal(recip[0:1, :], sums_sb[0:1, :])
                    ve.reciprocal(recip[64:65, :], sums_sb[64:65, :])
                    for i, h in enumerate(hs):
                        pbc = psBC.tile([P, TQ], f32, tag="pbc", name="pbc")
                        te.matmul(pbc[:], onesc[64 * i:64 * i + 1, 0:P],
                                  recip[64 * i:64 * i + 1, :],
                                  start=True, stop=True)
                        bc = sbBC.tile([P, TQ], f32, tag="rbc", name="rbc")
                        sc.activation(bc[:], pbc[:], Copy)
                        ve.tensor_mul(attnT[h][:, qsl], attnT[h][:, qsl],
                                      bc[:])

                # -- output projection for this group's 4 t-blocks --
                for tb in range(4 * gq, 4 * gq + 4):
                    ysb = sbD.tile([P, C], f32, tag="ysb", name="ysb")
                    for cc4 in range(C // 512):
                        py = psY.tile([P, 512], f32, tag="py", name="py")
                        for h in range(HL):
                            te.matmul(
                                py[:],
                                attnT[h][:, tb * P:(tb + 1) * P],
                                woT[h][:, cc4 * 512:(cc4 + 1) * 512],
                                start=(h == 0), stop=(h == HL - 1),
                            )
                        ve.tensor_copy(ysb[:, cc4 * 512:(cc4 + 1) * 512],
                                       py[:])
                    sy.dma_start(y_dram[tb * P:(tb + 1) * P, :], ysb[:])

                # -- reduce-scatter this group's rows; each core keeps 128 --
                gp.collective_compute(
                    "ReduceScatter", mybir.AluOpType.add,
                    replica_groups=[[0, 1, 2, 3], [4, 5, 6, 7]],
                    ins=[y_dram[gq * TQ:(gq + 1) * TQ, :].opt()],
                    outs=[rs_out[gq].opt()],
                )
                sy.dma_start(out.ap()[gq * P:(gq + 1) * P, :], rs_out[gq][:])

    return nc


_PROGRAM = None


def _get_program():
    global _PROGRAM
    if _PROGRAM is None:
        from contextlib import ExitStack
        import concourse.tile as tile
        from concourse import bacc, mybir

        nc = bacc.Bacc("TRN2", target_bir_lowering=False, debug=False,
                       num_devices=NCORES)
        _emit(nc, tile, mybir, ExitStack)
        nc.compile()
        _PROGRAM = nc
    return _PROGRAM


def kernel(x, wq, wk, wv, wo, freqs_cos, freqs_sin, mask=None):
    from concourse.bass_utils import run_bass_kernel_spmd

    x = np.asarray(x, np.float32)
    wq = np.asarray(wq, np.float32)
    wk = np.asarray(wk, np.float32)
    wv = np.asarray(wv, np.float32)
    wo = np.asarray(wo, np.float32)
    fc = np.ascontiguousarray(np.asarray(freqs_cos, np.float32))
    fs = np.ascontiguousarray(np.asarray(freqs_sin, np.float32))

    nc = _get_program()
    in_maps = []
    for core in range(NCORES):
        b, g = core // 4, core % 4
        in_maps.append({
            "xb": np.ascontiguousarray(x[b]),
            "wq": np.ascontiguousarray(wq[g * HL * D:(g + 1) * HL * D]),
            "wk": np.ascontiguousarray(wk[g * D:(g + 1) * D]),
            "wv": np.ascontiguousarray(wv[g * D:(g + 1) * D]),
            "wo": np.ascontiguousarray(wo[:, g * HL * D:(g + 1) * HL * D]),
            "fcos": fc,
            "fsin": fs,
        })
    res = run_bass_kernel_spmd(nc, in_maps, core_ids=list(range(NCORES)))
    outp = np.empty((B, T, C), np.float32)
    for b in range(B):
        for r in range(4):
            piece = res.results[4 * b + r]["out"]  # [NG*128, C]
            for gq in range(NG):
                outp[b, 512 * gq + 128 * r:512 * gq + 128 * (r + 1)] = \
                    piece[128 * gq:128 * (gq + 1)]
    return outp


# revision 18
# speedup vs baseline: 1.2037x; 1.0375x over previous
"""Trainium2 Bass kernel for GQA attention with RoPE (dense_transformer).

Model: B=2, T=2048, C=2048, H=16 query heads, KV=4 kv heads, D=128, causal.
Sharding: 8 cores = batch(2) x kv-group(4) tensor parallel. Each core computes
its batch's 4 query heads (one kv head), then a partial output projection over
its 512 head-dims; a 4-core ReduceScatter sums the partials and leaves each
core with a 512-row stripe of the final [2048, 2048] output.
"""

import os

os.environ.setdefault("MYCRO_LOCAL_CACHE", "1")

import numpy as np

_LDW_OPT_PATCHED = False


def _patch_ldw_opt():
    # walrus ships with LDWEIGHTS dedup disabled; consecutive matmuls that
    # reuse the same stationary operand benefit measurably from it here.
    global _LDW_OPT_PATCHED
    if _LDW_OPT_PATCHED:
        return
    import concourse.bass_utils as _bu
    _orig = _bu.run_command

    def _run(cmd, *a, **kw):
        if isinstance(cmd, list):
            cmd = ["--enable-ldw-opt=true" if c == "--enable-ldw-opt=false"
                   else c for c in cmd]
        return _orig(cmd, *a, **kw)

    _bu.run_command = _run
    _LDW_OPT_PATCHED = True


B, T, C = 2, 2048, 2048
H, KV, D = 16, 4, 128
HL = H // KV          # 4 local query heads per core
NCORES = 8
P = 128
SCALE = 1.0 / float(np.sqrt(D))

NT = T // P           # 16 t-blocks
NCC = C // P          # 16 c-chunks
NTC = T // 512        # 4 t-chunks of 512
NG = T // 512         # 4 q-block groups (512 queries each)
TQ = 512              # queries per attention group
NEG = -1e10


def _emit(nc, tile, mybir, ExitStack):
    from concourse.masks import make_identity

    f32 = mybir.dt.float32
    f32r = mybir.dt.float32r
    bf16 = mybir.dt.bfloat16
    Exp = mybir.ActivationFunctionType.Exp
    Copy = mybir.ActivationFunctionType.Copy
    add = mybir.AluOpType.add

    xb = nc.dram_tensor("xb", [T, C], f32, kind="ExternalInput")
    wq = nc.dram_tensor("wq", [HL * D, C], f32, kind="ExternalInput")
    wk = nc.dram_tensor("wk", [D, C], f32, kind="ExternalInput")
    wv = nc.dram_tensor("wv", [D, C], f32, kind="ExternalInput")
    wo = nc.dram_tensor("wo", [C, HL * D], f32, kind="ExternalInput")
    fcos = nc.dram_tensor("fcos", [T, D // 2], f32, kind="ExternalInput")
    fsin = nc.dram_tensor("fsin", [T, D // 2], f32, kind="ExternalInput")
    out = nc.dram_tensor("out", [T // 4, C], f32, kind="ExternalOutput")

    te, ve, sc, gp, sy = nc.tensor, nc.vector, nc.scalar, nc.gpsimd, nc.sync

    with tile.TileContext(nc) as tc, ExitStack() as ctx:
        consts = ctx.enter_context(tc.tile_pool(name="consts", bufs=1))
        persist = ctx.enter_context(tc.tile_pool(name="persist", bufs=1))
        dram = ctx.enter_context(tc.tile_pool(name="dram", bufs=1, space="DRAM"))

        ident = consts.tile([P, P], bf16, tag="ident")
        make_identity(nc, ident[:])
        # scoresT layout [tk, tq]: keep where tq >= tk, else -1e10.
        triT = consts.tile([P, P], f32, tag="triT")
        gp.memset(triT[:], 0.0)
        gp.affine_select(
            out=triT[:], in_=triT[:], compare_op=mybir.AluOpType.is_ge,
            fill=NEG, base=0, pattern=[[1, P]], channel_multiplier=-1,
        )
        ones = consts.tile([P, 1], bf16, tag="ones")
        gp.memset(ones[:], 1.0)
        onesc = consts.tile([P, P], f32, tag="onesc")
        gp.memset(onesc[:], 1.0)

        # tensors that live from projection until the end
        qrT = [persist.tile([P, T], bf16, tag=f"qrT{h}", name=f"qrT{h}")
               for h in range(HL)]
        krT = persist.tile([P, T], bf16, tag="krT")
        vnat = persist.tile([P, T], bf16, tag="vnat")
        attnT = [persist.tile([P, T], bf16, tag=f"attnT{h}", name=f"attnT{h}")
                 for h in range(HL)]

        y_dram = [dram.tile([TQ, C], f32, tag=f"ydram{g}", name=f"ydram{g}")
                  for g in range(NG)]
        rs_out = [dram.tile([P, C], f32, tag=f"rsout{g}", name=f"rsout{g}")
                  for g in range(NG)]

        # ---- projection phase (pools freed afterwards) ------------------------
        with tc.tile_pool(name="proj", bufs=1) as proj:
            cosq = proj.tile([P, T], bf16, tag="cosq")
            sinq = proj.tile([P, T], bf16, tag="sinq")
            cosk = proj.tile([P, T], bf16, tag="cosk")
            sink = proj.tile([P, T], bf16, tag="sink")
            xT = [proj.tile([P, T], bf16, tag=f"xT{cc}", name=f"xT{cc}")
                  for cc in range(NCC)]
            wqT = [proj.tile([P, HL * P], bf16, tag=f"wqT{cc}", name=f"wqT{cc}")
                   for cc in range(NCC)]
            wkT = [proj.tile([P, P], bf16, tag=f"wkT{cc}", name=f"wkT{cc}")
                   for cc in range(NCC)]
            wvT = [proj.tile([P, P], bf16, tag=f"wvT{cc}", name=f"wvT{cc}")
                   for cc in range(NCC)]
            vT = proj.tile([P, T], bf16, tag="vT")

            # -- stage A: loads, casts, transposes --
            with tc.tile_pool(name="stA", bufs=3) as sbA, \
                 tc.tile_pool(name="stAbf", bufs=5) as sbX, \
                 tc.tile_pool(name="psA", bufs=2, space="PSUM") as psA:

                # freqs -> cosT/sinT (bf16); q-versions pre-scaled by 1/sqrt(D)
                for src, dq, dk in ((fcos, cosq, cosk), (fsin, sinq, sink)):
                    for tb in range(NT):
                        ft = sbA.tile([P, 64], f32, tag="frq_in", name="frq_in")
                        sc.dma_start(ft[:], src.ap()[tb * P:(tb + 1) * P, :])
                        fb = sbA.tile([P, 64], bf16, tag="frq_bf", name="frq_bf")
                        ve.tensor_copy(fb[:], ft[:])
                        pf = psA.tile([64, P], bf16, tag="frq_ps", name="frq_ps")
                        te.transpose(pf[:], fb[:], ident[:])
                        sc.activation(dq[0:64, tb * P:(tb + 1) * P], pf[:],
                                      Copy, scale=SCALE)
                        sc.activation(dk[0:64, tb * P:(tb + 1) * P], pf[:],
                                      Copy)
                    # duplicate into the upper partition half
                    sy.dma_start(dq[64:P, :], dq[0:64, :])
                    sy.dma_start(dk[64:P, :], dk[0:64, :])

                # x -> xT (bf16)  [c-chunk][c_in_chunk, t]
                for tbg in range(NT // 4):
                    xbf = []
                    for i in range(4):
                        tb = tbg * 4 + i
                        xt = sbA.tile([P, C], f32, tag="big_in", name="big_in")
                        sy.dma_start(xt[:], xb.ap()[tb * P:(tb + 1) * P, :])
                        xc = sbX.tile([P, C], bf16, tag="big_bf", name="big_bf")
                        ve.tensor_copy(xc[:], xt[:])
                        xbf.append(xc)
                    for cc in range(NCC):
                        ps = psA.tile([P, 512], bf16, tag="ptr", name="ptr")
                        for i in range(4):
                            te.transpose(
                                ps[:, i * P:(i + 1) * P],
                                xbf[i][:, cc * P:(cc + 1) * P], ident[:],
                            )
                        sc.activation(
                            xT[cc][:, tbg * 512:(tbg + 1) * 512], ps[:], Copy)

                # wq (rows permuted even/odd per head) -> wqT
                wq_eo = wq.ap().rearrange("(a two) c -> two a c", two=2)
                wq_bf = []
                for h in range(HL):
                    wt = sbA.tile([P, C], f32, tag="big_in", name="big_in")
                    gp.dma_start(wt[0:64, :], wq_eo[0, h * 64:(h + 1) * 64, :])
                    gp.dma_start(wt[64:P, :], wq_eo[1, h * 64:(h + 1) * 64, :])
                    wb = sbX.tile([P, C], bf16, tag="big_bf", name="big_bf")
                    ve.tensor_copy(wb[:], wt[:])
                    wq_bf.append(wb)
                for cc in range(NCC):
                    ps = psA.tile([P, 512], bf16, tag="ptr", name="ptr")
                    for h in range(HL):
                        te.transpose(
                            ps[:, h * P:(h + 1) * P],
                            wq_bf[h][:, cc * P:(cc + 1) * P], ident[:],
                        )
                    sc.activation(wqT[cc][:], ps[:], Copy)

                # wk (permuted) / wv (natural) -> wkT / wvT
                wk_eo = wk.ap().rearrange("(a two) c -> two a c", two=2)
                for src_eo, src, dst, perm in (
                        (wk_eo, wk, wkT, True), (None, wv, wvT, False)):
                    wt = sbA.tile([P, C], f32, tag="big_in", name="big_in")
                    if perm:
                        gp.dma_start(wt[0:64, :], src_eo[0, :, :])
                        gp.dma_start(wt[64:P, :], src_eo[1, :, :])
                    else:
                        gp.dma_start(wt[:], src.ap()[:, :])
                    wb = sbX.tile([P, C], bf16, tag="big_bf", name="big_bf")
                    ve.tensor_copy(wb[:], wt[:])
                    for ccg in range(NCC // 4):
                        ps = psA.tile([P, 512], bf16, tag="ptr", name="ptr")
                        for i in range(4):
                            cc = ccg * 4 + i
                            te.transpose(
                                ps[:, i * P:(i + 1) * P],
                                wb[:, cc * P:(cc + 1) * P], ident[:],
                            )
                        for i in range(4):
                            cc = ccg * 4 + i
                            sc.activation(dst[cc][:], ps[:, i * P:(i + 1) * P],
                                          Copy)

            # -- stage B: QKV projections + rope --
            with tc.tile_pool(name="stB", bufs=4) as sbB, \
                 tc.tile_pool(name="ropetmp", bufs=8) as sbR, \
                 tc.tile_pool(name="psB", bufs=2, space="PSUM") as psB:

                def qkv_unit(weight_slices, dst, rope, cos_t, sin_t):
                    for tc4 in range(NTC):
                        ps = psB.tile([P, 512], f32, tag="pqkv", name="pqkv")
                        for cc in range(NCC):
                            te.matmul(
                                ps[:], weight_slices[cc],
                                xT[cc][:, tc4 * 512:(tc4 + 1) * 512],
                                start=(cc == 0), stop=(cc == NCC - 1),
                            )
                        sl = slice(tc4 * 512, (tc4 + 1) * 512)
                        if not rope:
                            sc.activation(dst[:, sl], ps[:], Copy)
                            continue
                        qs = sbB.tile([P, 512], bf16, tag="qkev", name="qkev")
                        sc.activation(qs[:], ps[:], Copy)
                        # realign halves so every DVE op is base-consistent
                        q1lo = sbR.tile([64, 512], bf16, tag="q1lo", name="q1lo")
                        sy.dma_start(q1lo[:], qs[64:P, :])
                        q0hi = sbR.tile([P, 512], bf16, tag="q0hi", name="q0hi")
                        sy.dma_start(q0hi[64:P, :], qs[0:64, :])
                        ta = sbR.tile([64, 512], bf16, tag="rta", name="rta")
                        tb2 = sbR.tile([64, 512], bf16, tag="rtb", name="rtb")
                        ve.tensor_mul(ta[:], qs[0:64, :], cos_t[0:64, sl])
                        ve.tensor_mul(tb2[:], q1lo[:], sin_t[0:64, sl])
                        ve.tensor_sub(dst[0:64, sl], ta[:], tb2[:])
                        tc2 = sbR.tile([P, 512], bf16, tag="rtc", name="rtc")
                        td = sbR.tile([P, 512], bf16, tag="rtd", name="rtd")
                        ve.tensor_mul(tc2[64:P, :], q0hi[64:P, :], sin_t[64:P, sl])
                        ve.tensor_mul(td[64:P, :], qs[64:P, :], cos_t[64:P, sl])
                        ve.tensor_add(dst[64:P, sl], tc2[64:P, :], td[64:P, :])

                for h in range(HL):
                    qkv_unit(
                        [wqT[cc][:, h * P:(h + 1) * P] for cc in range(NCC)],
                        qrT[h], True, cosq, sinq,
                    )
                qkv_unit([wkT[cc][:] for cc in range(NCC)], krT, True,
                         cosk, sink)
                qkv_unit([wvT[cc][:] for cc in range(NCC)], vT, False,
                         None, None)

                # vT [dv, t] -> vnat [partition=t%128, col = kb*128 + dv]
                for tbg in range(NT // 4):
                    ps = psB.tile([P, 512], bf16, tag="pvtr", name="pvtr")
                    for i in range(4):
                        tb = tbg * 4 + i
                        te.transpose(
                            ps[:, i * P:(i + 1) * P],
                            vT[:, tb * P:(tb + 1) * P], ident[:],
                        )
                    sc.activation(vnat[:, tbg * 512:(tbg + 1) * 512], ps[:],
                                  Copy)

        # ---- attention + output projection phase ------------------------------
        with tc.tile_pool(name="stCD", bufs=1) as sbCD, \
             tc.tile_pool(name="stCDin", bufs=2) as sbCin, \
             tc.tile_pool(name="stCDbf", bufs=4) as sbCbf, \
             tc.tile_pool(name="probsP", bufs=6) as sbP, \
             tc.tile_pool(name="stC", bufs=2) as sbC, \
             tc.tile_pool(name="stCbc", bufs=4) as sbBC, \
             tc.tile_pool(name="stD", bufs=3) as sbD, \
             tc.tile_pool(name="psWo", bufs=1, space="PSUM") as psWo, \
             tc.tile_pool(name="psT", bufs=2, space="PSUM") as psT, \
             tc.tile_pool(name="psAttn", bufs=2, space="PSUM") as psAt, \
             tc.tile_pool(name="psSums", bufs=1, space="PSUM") as psSm, \
             tc.tile_pool(name="psY", bufs=1, space="PSUM") as psY, \
             tc.tile_pool(name="psBC", bufs=1, space="PSUM") as psBC:

            # wo [C, HL*D] -> woT[h] [dv, C]  (overlaps attention on PE gaps)
            woT = [sbCD.tile([P, C], bf16, tag=f"woT{h}", name=f"woT{h}")
                   for h in range(HL)]
            for ctg in range(NCC // 4):
                wo_bf = []
                for i in range(4):
                    ct = ctg * 4 + i
                    wt = sbCin.tile([P, HL * P], f32, tag="wo_in", name="wo_in")
                    gp.dma_start(wt[:], wo.ap()[ct * P:(ct + 1) * P, :])
                    wb = sbCbf.tile([P, HL * P], bf16, tag="wo_bf", name="wo_bf")
                    ve.tensor_copy(wb[:], wt[:])
                    wo_bf.append(wb)
                for h in range(HL):
                    ps = psWo.tile([P, 512], bf16, tag="pwo", name="pwo")
                    for i in range(4):
                        te.transpose(
                            ps[:, i * P:(i + 1) * P],
                            wo_bf[i][:, h * P:(h + 1) * P], ident[:],
                        )
                    sc.activation(woT[h][:, ctg * 512:(ctg + 1) * 512], ps[:],
                                  Copy)

            # attention, two heads per sweep to limit PSUM pressure;
            # per-group output projection + reduce-scatter overlap the rest
            for gq in range(NG):
                kbmax = 4 * (gq + 1)
                qsl = slice(gq * TQ, (gq + 1) * TQ)
                for hp in range(HL // 2):
                    hs = (2 * hp, 2 * hp + 1)
                    pa = [psAt.tile([P, TQ], f32, tag="pattn", name="pattn")
                          for _ in hs]
                    psums = psSm.tile([P, TQ], f32, tag="psums", name="psums")
                    for kb in range(kbmax):
                        j = kb - 4 * gq  # >= 0 on diagonal blocks
                        w0 = max(j, 0) * P
                        probs = []
                        for h in hs:
                            st = psT.tile([P, TQ], f32, tag="pscore",
                                          name="pscore")
                            te.matmul(
                                st[:, w0:TQ],
                                krT[:, kb * P:(kb + 1) * P],
                                qrT[h][:, gq * TQ + w0:(gq + 1) * TQ],
                                start=True, stop=True,
                            )
                            if j >= 0:
                                ve.tensor_tensor(
                                    st[:, w0:w0 + P], st[:, w0:w0 + P],
                                    triT[:], add)
                            pb = sbP.tile([P, TQ], bf16, tag="probs",
                                          name="probs")
                            sc.activation(pb[:, w0:TQ], st[:, w0:TQ], Exp)
                            if w0 > 0:
                                gp.memset(pb[:, 0:w0], 0.0)
                            probs.append(pb)
                        for i in range(2):
                            te.matmul(
                                psums[64 * i:64 * i + 1, :], ones[:],
                                probs[i][:],
                                start=(kb == 0), stop=(kb == kbmax - 1),
                            )
                        for i in range(2):
                            te.matmul(
                                pa[i][:], vnat[:, kb * P:(kb + 1) * P],
                                probs[i][:],
                                start=(kb == 0), stop=(kb == kbmax - 1),
                            )
                    # evict unnormalized immediately (frees PSUM for the next
                    # sweep), normalize off the critical path
                    sums_sb = sbC.tile([P, TQ], f32, tag="sums_sb",
                                       name="sums_sb")
                    sc.activation(sums_sb[0:1, :], psums[0:1, :], Copy)
                    sc.activation(sums_sb[64:65, :], psums[64:65, :], Copy)
                    for i, h in enumerate(hs):
                        sc.activation(attnT[h][:, qsl], pa[i][:], Copy)
                    recip = sbC.tile([P, TQ], f32, tag="recip", name="recip")
                    ve.reciprocal(recip[0:1, :], sums_sb[0:1, :])
                    ve.reciprocal(recip[64:65, :], sums_sb[64:65, :])
                    for i, h in enumerate(hs):
                        pbc = psBC.tile([P, TQ], f32, tag="pbc", name="pbc")
                        te.matmul(pbc[:], onesc[64 * i:64 * i + 1, 0:P],
                                  recip[64 * i:64 * i + 1, :],
                                  start=True, stop=True)
                        bc = sbBC.tile([P, TQ], f32, tag="rbc", name="rbc")
                        sc.activation(bc[:], pbc[:], Copy)
                        ve.tensor_mul(attnT[h][:, qsl], attnT[h][:, qsl],
                                      bc[:])

                # -- output projection for this group's 4 t-blocks --
                for tb in range(4 * gq, 4 * gq + 4):
                    ysb = sbD.tile([P, C], f32, tag="ysb", name="ysb")
                    for cc4 in range(C // 512):
                        py = psY.tile([P, 512], f32, tag="py", name="py")
                        for h in range(HL):
                            te.matmul(
                                py[:],
                                attnT[h][:, tb * P:(tb + 1) * P],
                                woT[h][:, cc4 * 512:(cc4 + 1) * 512],
                                start=(h == 0), stop=(h == HL - 1),
                            )
                        ve.tensor_copy(ysb[:, cc4 * 512:(cc4 + 1) * 512],
                                       py[:])
                    sy.dma_start(
                        y_dram[gq][(tb - 4 * gq) * P:(tb - 4 * gq + 1) * P, :],
                        ysb[:])

                # -- reduce-scatter this group's rows; each core keeps 128 --
                gp.collective_compute(
                    "ReduceScatter", mybir.AluOpType.add,
                    replica_groups=[[0, 1, 2, 3], [4, 5, 6, 7]],
                    ins=[y_dram[gq][:, :].opt()],
                    outs=[rs_out[gq].opt()],
                )
                sy.dma_start(out.ap()[gq * P:(gq + 1) * P, :], rs_out[gq][:])

    return nc


_PROGRAM = None


def _get_program():
    global _PROGRAM
    if _PROGRAM is None:
        from contextlib import ExitStack
        import concourse.tile as tile
        from concourse import bacc, mybir

        nc = bacc.Bacc("TRN2", target_bir_lowering=False, debug=False,
                       num_devices=NCORES)
        _emit(nc, tile, mybir, ExitStack)
        nc.compile()
        _PROGRAM = nc
    return _PROGRAM


def kernel(x, wq, wk, wv, wo, freqs_cos, freqs_sin, mask=None):
    from concourse.bass_utils import run_bass_kernel_spmd

    x = np.asarray(x, np.float32)
    wq = np.asarray(wq, np.float32)
    wk = np.asarray(wk, np.float32)
    wv = np.asarray(wv, np.float32)
    wo = np.asarray(wo, np.float32)
    fc = np.ascontiguousarray(np.asarray(freqs_cos, np.float32))
    fs = np.ascontiguousarray(np.asarray(freqs_sin, np.float32))

    nc = _get_program()
    in_maps = []
    for core in range(NCORES):
        b, g = core // 4, core % 4
        in_maps.append({
            "xb": np.ascontiguousarray(x[b]),
            "wq": np.ascontiguousarray(wq[g * HL * D:(g + 1) * HL * D]),
            "wk": np.ascontiguousarray(wk[g * D:(g + 1) * D]),
            "wv": np.ascontiguousarray(wv[g * D:(g + 1) * D]),
            "wo": np.ascontiguousarray(wo[:, g * HL * D:(g + 1) * HL * D]),
            "fcos": fc,
            "fsin": fs,
        })
    res = run_bass_kernel_spmd(nc, in_maps, core_ids=list(range(NCORES)))
    outp = np.empty((B, T, C), np.float32)
    for b in range(B):
        for r in range(4):
            piece = res.results[4 * b + r]["out"]  # [NG*128, C]
            for gq in range(NG):
                outp[b, 512 * gq + 128 * r:512 * gq + 128 * (r + 1)] = \
                    piece[128 * gq:128 * (gq + 1)]
    return outp
